# revision 1
# baseline (speedup 1.0000x reference)
"""CAM (channel attention) module kernel for Trainium2, data-parallel over batch.

Computes, per sample:
    v = x.reshape(C, N)                  # N = H*W
    energy = v @ v.T                     # [C, C]
    att = softmax(rowmax(energy) - energy, axis=-1)
    out = gamma * (att @ v) + x

Distribution: batch B=32 split over 8 NeuronCores (4 samples/core), gamma
replicated.  Per core everything is computed on-chip:
  - v loaded once to SBUF (doubles as x for the residual add)
  - v^T built with PE transpose-mode matmuls (needed for the energy matmul,
    whose contraction is over the spatial axis)
  - energy accumulated in PSUM with float32r (FP22) matmuls: full bf16-rate
    (1 cycle/row) with ~2^-12 operand rounding, which end-to-end gives
    ~1e-3 max error (validated numerically)
  - softmax via the identity softmax(rowmax - e) = exp(rowmin - e)/sum(...):
    row-min on DVE, exp (+ fused row-sum) on ACT
  - unnormalized attention transposed with 16 PE transposes, so the second
    matmul can contract over the attention column axis; the row
    normalization (1/Z) and gamma are folded into a single per-partition
    scalar applied in the epilogue
  - epilogue fuses (psum * (gamma/Z)) + x in one DVE pass
"""

import sys

sys.path.insert(0, "/opt/trn_rl_repo")

from contextlib import ExitStack

import numpy as np

import concourse.bacc as bacc
import concourse.bass as bass
import concourse.mybir as mybir
import concourse.tile as tile
from concourse import masks
from concourse.bass_utils import run_bass_kernel_spmd

B, C, H, W = 32, 512, 48, 48
N = H * W  # 2304
NCORES = 8
SPC = B // NCORES  # samples per core
P = 128
CB = C // P  # 4 channel blocks
KB = N // P  # 18 spatial chunks of 128
NCH = [512, 512, 512, 512, 256]  # free-dim chunking of N for the 2nd matmul

FP32 = mybir.dt.float32
FP32R = mybir.dt.float32r
AX = mybir.AxisListType.X
OP = mybir.AluOpType
AF = mybir.ActivationFunctionType


def _emit(tc, ctx, x, gamma, out, reps=1):
    nc = tc.nc

    const_pool = ctx.enter_context(tc.tile_pool(name="const", bufs=1))
    ident_f32 = const_pool.tile([P, P], FP32)
    masks.make_identity(nc, ident_f32[:])
    ident = const_pool.tile([P, P], FP32R)
    nc.scalar.copy(ident[:], ident_f32[:])
    gamma_sb = const_pool.tile([P, 1], FP32)
    nc.sync.dma_start(gamma_sb[:], bass.AP(gamma.tensor, 0, [[0, P], [1, 1]]))

    v_pool = ctx.enter_context(tc.tile_pool(name="v", bufs=3))
    vt_pool = ctx.enter_context(tc.tile_pool(name="vt", bufs=1))
    p_pool = ctx.enter_context(tc.tile_pool(name="p", bufs=2))
    pt_pool = ctx.enter_context(tc.tile_pool(name="pt", bufs=2))
    o_pool = ctx.enter_context(tc.tile_pool(name="o", bufs=3))
    vec_pool = ctx.enter_context(tc.tile_pool(name="vec", bufs=4))
    s_pool = ctx.enter_context(tc.tile_pool(name="s", bufs=2))
    # PSUM budget is exactly 8 banks: energy/attn-T share one 4-bank slot
    # (their lifetimes are disjoint), 2 rotating transpose banks, 2 output
    # banks.
    ps_e = ctx.enter_context(tc.tile_pool(name="ps_e", bufs=1, space="PSUM"))
    ps_t = ctx.enter_context(tc.tile_pool(name="ps_t", bufs=2, space="PSUM"))
    ps_o = ctx.enter_context(tc.tile_pool(name="ps_o", bufs=2, space="PSUM"))

    nsamp = reps * SPC
    v_t = {}
    vt_t = {}

    def load_v(i):
        # split per c-block into 3 column ranges so the first transposes can
        # start before the whole sample has landed
        s = i % SPC
        v = v_pool.tile([P, CB * N], FP32R, tag="v", name=f"v{i}")
        for cb in range(CB):
            for a, b in ((0, 768), (768, 1536), (1536, N)):
                nc.sync.dma_start(
                    v[:, cb * N + a : cb * N + b],
                    x[s, cb * P : (cb + 1) * P, a:b].bitcast(FP32R),
                )
        v_t[i] = v

    def a_chunk(i, k, copy_eng=None):
        # transpose one 128-wide spatial chunk of v into vt
        if k == 0:
            vt_t[i] = vt_pool.tile([P, KB * C], FP32R, tag="vt", name=f"vt{i}")
        v, vt = v_t[i], vt_t[i]
        tps = ps_t.tile([P, 512], FP32R, tag="tps")
        for cb in range(CB):
            nc.tensor.matmul(
                tps[:, cb * P : (cb + 1) * P],
                v[:, cb * N + k * P : cb * N + (k + 1) * P],
                ident[:],
                is_transpose=True,
                start=(cb == 0),
                stop=(cb == CB - 1),
            )
        if copy_eng == "dve":
            nc.vector.tensor_copy(vt[:, k * C : (k + 1) * C], tps[:])
        elif copy_eng == "act":
            nc.scalar.copy(vt[:, k * C : (k + 1) * C], tps[:])
        else:
            nc.any.tensor_copy(vt[:, k * C : (k + 1) * C], tps[:])

    def emit(i):
        # ---- energy = v v^T (ib-outer so softmax overlaps later blocks) ----
        s = i % SPC
        v, vt = v_t[i], vt_t.get(i)
        if i + 1 < nsamp:
            load_v(i + 1)
        energy = ps_e.tile([P, CB * 512], FP32, tag="eb")
        p_sb = p_pool.tile([P, CB * 512], FP32R, tag="p")
        s_all = s_pool.tile([P, CB], FP32, tag="s")
        for ib in range(CB):
            for k in range(KB):
                if i == 0 and ib == 0:
                    # prologue: sample 0 has no previous sample to hide its
                    # transposes under — build each chunk just-in-time
                    if k == 0:
                        a_chunk(0, 0)
                    if k + 1 < KB:
                        a_chunk(0, k + 1)
                    vt = vt_t[0]
                nc.tensor.matmul(
                    energy[:, ib * 512 : (ib + 1) * 512],
                    vt[:, k * C + ib * P : k * C + (ib + 1) * P],
                    vt[:, k * C : (k + 1) * C],
                    start=(k == 0),
                    stop=(k == KB - 1),
                )
            # softmax(rowmax - e) == exp(rowmin - e) / rowsum
            e_ib = energy[:, ib * 512 : (ib + 1) * 512]
            mn = vec_pool.tile([P, 1], FP32, tag="mn")
            nc.vector.tensor_reduce(mn[:], e_ib, axis=AX, op=OP.min)
            z = vec_pool.tile([P, 1], FP32, tag="z")
            nc.scalar.activation(
                p_sb[:, ib * 512 : (ib + 1) * 512],
                e_ib,
                AF.Exp,
                bias=mn[:],
                scale=-1.0,
                accum_out=z[:],
            )
            r = vec_pool.tile([P, 1], FP32, tag="r")
            nc.vector.reciprocal(r[:], z[:])
            nc.vector.tensor_tensor(
                s_all[:, ib : ib + 1], r[:], gamma_sb[:], op=OP.mult
            )

        # a few of the next sample's transposes fill the exp tail (copies on
        # DVE: ACT is busy with the exps here)
        if i + 1 < nsamp:
            a_queue = list(range(KB))
            for _ in range(3):
                a_chunk(i + 1, a_queue.pop(0), copy_eng="dve")
        else:
            a_queue = []

        # ---- transpose unnormalized attention: PT[d, c] = P[c, d] ----
        # reuses the energy banks (tag "eb"): 16 blocks, one group per bank
        pt_ps = ps_e.tile([P, CB * 512], FP32R, tag="eb")
        for cb in range(CB):
            for db in range(CB):
                nc.tensor.matmul(
                    pt_ps[:, db * 512 + cb * P : db * 512 + (cb + 1) * P],
                    p_sb[:, cb * 512 + db * P : cb * 512 + (db + 1) * P],
                    ident[:],
                    is_transpose=True,
                    start=(cb == 0),
                    stop=(cb == CB - 1),
                )
        pt_sb = pt_pool.tile([P, CB * 512], FP32R, tag="pt")
        for db in range(CB):
            nc.vector.tensor_copy(
                pt_sb[:, db * 512 : (db + 1) * 512],
                pt_ps[:, db * 512 : (db + 1) * 512],
            )

        # ---- out = (PT^T @ v) * (gamma/Z) + x, next-sample transposes mixed in
        for cb in range(CB):
            n_off = 0
            for nch in NCH:
                if a_queue:
                    # copies on ACT: DVE is busy with the epilogue here
                    a_chunk(i + 1, a_queue.pop(0), copy_eng="act")
                po = ps_o.tile([P, 512], FP32, tag="po")
                for db in range(CB):
                    nc.tensor.matmul(
                        po[:, :nch],
                        pt_sb[:, db * 512 + cb * P : db * 512 + (cb + 1) * P],
                        v[:, db * N + n_off : db * N + n_off + nch],
                        start=(db == 0),
                        stop=(db == CB - 1),
                    )
                ot = o_pool.tile([P, 512], FP32, tag="ot")
                nc.vector.scalar_tensor_tensor(
                    ot[:, :nch],
                    po[:, :nch],
                    s_all[:, cb : cb + 1],
                    v[:, cb * N + n_off : cb * N + n_off + nch].bitcast(FP32),
                    op0=OP.mult,
                    op1=OP.add,
                )
                nc.sync.dma_start(
                    out[s, cb * P : (cb + 1) * P, n_off : n_off + nch], ot[:, :nch]
                )
                n_off += nch
        del v_t[i], vt_t[i]

    load_v(0)
    for i in range(nsamp):
        emit(i)


_nc_cache = {}


def _build(reps=1):
    if reps in _nc_cache:
        return _nc_cache[reps]
    nc = bacc.Bacc("TRN2", target_bir_lowering=False, debug=False)
    x_d = nc.dram_tensor("x", [SPC, C, N], FP32, kind="ExternalInput")
    g_d = nc.dram_tensor("gamma", [1], FP32, kind="ExternalInput")
    o_d = nc.dram_tensor("out", [SPC, C, N], FP32, kind="ExternalOutput")
    with tile.TileContext(nc) as tc, ExitStack() as ctx:
        _emit(tc, ctx, x_d.ap(), g_d.ap(), o_d.ap(), reps=reps)
    nc.compile()
    _nc_cache[reps] = nc
    return nc


def _bench_fn(reps, x, gamma):
    """Build a jitted 8-core executor for the reps-times-repeated kernel with
    device-resident inputs.  Used by test.py for differential timing."""
    import jax
    from jax.experimental.shard_map import shard_map
    from jax.sharding import Mesh, NamedSharding, PartitionSpec

    from concourse import bass2jax

    bass2jax.install_neuronx_cc_hook()
    nc = _build(reps=reps)
    pid = nc.partition_id_tensor.name if nc.partition_id_tensor else None
    in_names, out_names, out_avals, zero_outs = [], [], [], []
    for alloc in nc.m.functions[0].allocations:
        if not isinstance(alloc, mybir.MemoryLocationSet):
            continue
        name = alloc.memorylocations[0].name
        if alloc.kind == "ExternalInput":
            if name != pid:
                in_names.append(name)
        elif alloc.kind == "ExternalOutput":
            out_names.append(name)
            shape = tuple(alloc.tensor_shape)
            dtype = mybir.dt.np(alloc.dtype)
            out_avals.append(jax.core.ShapedArray(shape, dtype))
            zero_outs.append(np.zeros(shape, dtype))
    all_in_names = list(in_names) + list(out_names)
    if pid:
        all_in_names.append(pid)

    def _body(*args):
        operands = list(args)
        if pid:
            operands.append(bass2jax.partition_id_tensor())
        return tuple(
            bass2jax._bass_exec_p.bind(
                *operands,
                out_avals=tuple(out_avals),
                in_names=tuple(all_in_names),
                out_names=tuple(out_names),
                lowering_input_output_aliases=(),
                sim_require_finite=True,
                sim_require_nnan=True,
                nc=nc,
            )
        )

    devices = jax.devices()[:NCORES]
    mesh = Mesh(np.asarray(devices), ("core",))
    specs = (PartitionSpec("core"),) * (len(in_names) + len(out_names))
    fn = jax.jit(
        shard_map(
            _body,
            mesh=mesh,
            in_specs=specs,
            out_specs=(PartitionSpec("core"),) * len(out_names),
            check_rep=False,
        ),
        keep_unused=True,
    )
    sh = NamedSharding(mesh, PartitionSpec("core"))
    ins = {
        "x": np.ascontiguousarray(x, dtype=np.float32).reshape(B, C, N),
        "gamma": np.tile(np.ascontiguousarray(gamma, dtype=np.float32), (NCORES,)),
    }
    args = [jax.device_put(ins[n], sh) for n in in_names]
    args += [
        jax.device_put(np.zeros((NCORES * z.shape[0], *z.shape[1:]), z.dtype), sh)
        for z in zero_outs
    ]
    return fn, args


def kernel(x: np.ndarray, gamma: np.ndarray, **run_kwargs) -> np.ndarray:
    assert x.shape == (B, C, H, W), x.shape
    nc = _build()
    xr = np.ascontiguousarray(x, dtype=np.float32).reshape(B, C, N)
    g = np.ascontiguousarray(gamma, dtype=np.float32)
    in_maps = [
        {"x": xr[g_idx * SPC : (g_idx + 1) * SPC], "gamma": g}
        for g_idx in range(NCORES)
    ]
    res = run_bass_kernel_spmd(nc, in_maps, core_ids=list(range(NCORES)), **run_kwargs)
    outs = [res.results[g_idx]["out"] for g_idx in range(NCORES)]
    full = np.concatenate(outs, axis=0).reshape(B, C, H, W).astype(np.float32)
    if run_kwargs:
        kernel.last_results = res
    return full



# revision 3
# speedup vs baseline: 1.6344x; 1.6344x over previous
"""CAM (channel attention) module kernel for Trainium2, data-parallel over batch.

Computes, per sample:
    v = x.reshape(C, N)                  # N = H*W
    energy = v @ v.T                     # [C, C]
    att = softmax(rowmax(energy) - energy, axis=-1)
    out = gamma * (att @ v) + x

Distribution: batch B=32 split over 8 NeuronCores (4 samples/core), gamma
replicated.  Per core everything is computed on-chip:
  - v loaded once to SBUF (doubles as x for the residual add)
  - v^T built with PE transpose-mode matmuls (needed for the energy matmul,
    whose contraction is over the spatial axis)
  - energy is SYMMETRIC (E = V V^T), so only the upper-triangle block rows
    are computed (free widths 512/384/256/256 per 128-row block), with
    float32r (FP22) matmuls accumulated in PSUM
  - softmax via a GLOBAL-bias exponent: with any per-sample constant G,
    Q = exp(G - E) is symmetric, so Q doubles as the *transposed*
    unnormalized attention (the per-row max/min bias of the reference
    softmax cancels exactly against the row normalizer).  Row sums of Q
    (ACT accum_out) give Z'.  Then out = (Q @ v) * (gamma / Z') + x.
    This removes the attention transpose + per-row min reductions of the
    direct formulation entirely.
  - G is derived on-chip from block-0's row minima: G = (min+max)/2 - 10,
    computed with one DVE row-reduce + two gpsimd partition_all_reduce ops
    (numerics: exponents stay within +-83, Q in [1e-36, 2e36], validated)
  - the 5 lower-triangle Q blocks are mirrored from the upper ones with PE
    transposes; the DVE copy back to SBUF also row-sums them (accum_out) to
    complete Z'
  - epilogue fuses (psum * (gamma/Z')) + x in one DVE pass
"""

import sys

sys.path.insert(0, "/opt/trn_rl_repo")

from contextlib import ExitStack

import numpy as np

import concourse.bacc as bacc
import concourse.bass as bass
import concourse.bass_isa as bass_isa
import concourse.mybir as mybir
import concourse.tile as tile
from concourse import masks
from concourse.bass_utils import run_bass_kernel_spmd

B, C, H, W = 32, 512, 48, 48
N = H * W  # 2304
NCORES = 8
SPC = B // NCORES  # samples per core
P = 128
CB = C // P  # 4 channel blocks
KB = N // P  # 18 spatial chunks of 128
NCH = [512, 512, 512, 512, 256]  # free-dim chunking of N for the 2nd matmul
NRNG = 6  # column ranges per v load (range-major so transposes start early)
G_SHIFT = 10.0  # centers exp(G - E) in fp32 range (see module docstring)

# stored (upper-triangle) energy row-block geometry: row ib holds cols
# [CLO[ib]*P, 512) of the attention matrix
CLO = [0, 1, 2, 2]  # first stored 128-col block per row (row 3 keeps 2 blocks
#                     so the matmul free dim stays >= 256, the fp32r fast path)
MIRROR = [(1, 0), (2, 0), (2, 1), (3, 0), (3, 1)]  # (ib, jb): fill from (jb, ib)^T

FP32 = mybir.dt.float32
FP32R = mybir.dt.float32r
AX = mybir.AxisListType.X
OP = mybir.AluOpType
AF = mybir.ActivationFunctionType
RED = bass_isa.ReduceOp


def _emit(tc, ctx, x, gamma, out, reps=1):
    nc = tc.nc

    const_pool = ctx.enter_context(tc.tile_pool(name="const", bufs=1))
    ident_f32 = const_pool.tile([P, P], FP32)
    masks.make_identity(nc, ident_f32[:])
    ident = const_pool.tile([P, P], FP32R)
    nc.scalar.copy(ident[:], ident_f32[:])
    gamma_sb = const_pool.tile([P, 1], FP32)
    nc.sync.dma_start(gamma_sb[:], bass.AP(gamma.tensor, 0, [[0, P], [1, 1]]))

    v_pool = ctx.enter_context(tc.tile_pool(name="v", bufs=3))
    vt_pool = ctx.enter_context(tc.tile_pool(name="vt", bufs=1))
    q_pool = ctx.enter_context(tc.tile_pool(name="q", bufs=2))
    o_pool = ctx.enter_context(tc.tile_pool(name="o", bufs=3))
    vec_pool = ctx.enter_context(tc.tile_pool(name="vec", bufs=4))
    s_pool = ctx.enter_context(tc.tile_pool(name="s", bufs=2))
    # PSUM budget exactly 8 banks: 4 energy (one per row block), 2 rotating
    # transpose banks (v^T chunks and Q mirrors), 2 output banks.
    ps_e = ctx.enter_context(tc.tile_pool(name="ps_e", bufs=1, space="PSUM"))
    ps_t = ctx.enter_context(tc.tile_pool(name="ps_t", bufs=2, space="PSUM"))
    ps_o = ctx.enter_context(tc.tile_pool(name="ps_o", bufs=2, space="PSUM"))

    nsamp = reps * SPC
    v_t = {}
    vt_t = {}

    def load_v(i):
        # range-major issue order so the k-th transpose chunk only waits for
        # the first ceil((k+1)/3) ranges of each c-block row
        s = i % SPC
        v = v_pool.tile([P, CB * N], FP32R, tag="v", name=f"v{i}")
        rw = N // NRNG
        for r in range(NRNG):
            a, b = r * rw, (r + 1) * rw
            for cb in range(CB):
                nc.sync.dma_start(
                    v[:, cb * N + a : cb * N + b],
                    x[s, cb * P : (cb + 1) * P, a:b].bitcast(FP32R),
                )
        v_t[i] = v

    def a_chunk(i, k, copy_eng=None):
        # transpose one 128-wide spatial chunk of v into vt
        if k == 0:
            vt_t[i] = vt_pool.tile([P, KB * C], FP32R, tag="vt", name=f"vt{i}")
        v, vt = v_t[i], vt_t[i]
        tps = ps_t.tile([P, 512], FP32R, tag="tps")
        for cb in range(CB):
            nc.tensor.matmul(
                tps[:, cb * P : (cb + 1) * P],
                v[:, cb * N + k * P : cb * N + (k + 1) * P],
                ident[:],
                is_transpose=True,
                start=(cb == 0),
                stop=(cb == CB - 1),
            )
        if copy_eng == "dve":
            nc.vector.tensor_copy(vt[:, k * C : (k + 1) * C], tps[:])
        elif copy_eng == "act":
            nc.scalar.copy(vt[:, k * C : (k + 1) * C], tps[:])
        else:
            nc.any.tensor_copy(vt[:, k * C : (k + 1) * C], tps[:])

    def emit(i):
        s = i % SPC
        v, vt = v_t[i], vt_t.get(i)
        if i + 1 < nsamp:
            load_v(i + 1)

        # ---- upper-triangle energy: row block ib holds cols [CLO[ib]*P, 512)
        energy = ps_e.tile([P, CB * 512], FP32, tag="eb")
        q_sb = q_pool.tile([P, CB * 512], FP32R, tag="q")
        z = vec_pool.tile([P, CB], FP32, tag="z")
        s_all = s_pool.tile([P, CB], FP32, tag="s")

        def row_rgn(ib):
            return slice(ib * 512 + CLO[ib] * P, (ib + 1) * 512)

        for ib in range(CB):
            lo = CLO[ib] * P
            for k in range(KB):
                if i == 0 and ib == 0:
                    # prologue: sample 0 builds vt chunks just-in-time
                    if k == 0:
                        a_chunk(0, 0)
                    if k + 1 < KB:
                        a_chunk(0, k + 1)
                    vt = vt_t[0]
                nc.tensor.matmul(
                    energy[:, row_rgn(ib)],
                    vt[:, k * C + ib * P : k * C + (ib + 1) * P],
                    vt[:, k * C + lo : k * C + C],
                    start=(k == 0),
                    stop=(k == KB - 1),
                )
            if ib == 0:
                # ---- global bias G = (min+max of block-0 row minima)/2 - 10
                mn0 = vec_pool.tile([P, 1], FP32, tag="mn0")
                nc.vector.tensor_reduce(
                    mn0[:], energy[:, 0:512], axis=AX, op=OP.min
                )
                nm0 = vec_pool.tile([P, 1], FP32, tag="nm0")
                nc.vector.tensor_scalar(nm0[:], mn0[:], -1.0, None, OP.mult)
                mx_r = vec_pool.tile([P, 1], FP32, tag="mx_r")
                nc.gpsimd.partition_all_reduce(mx_r[:], mn0[:], P, RED.max)
                nmn_r = vec_pool.tile([P, 1], FP32, tag="nmn_r")
                nc.gpsimd.partition_all_reduce(nmn_r[:], nm0[:], P, RED.max)
                g_sb = vec_pool.tile([P, 1], FP32, tag="g_sb")
                nc.vector.tensor_tensor(g_sb[:], mx_r[:], nmn_r[:], op=OP.subtract)
                nc.vector.tensor_scalar(
                    g_sb[:], g_sb[:], 0.5, -G_SHIFT, OP.mult, OP.add
                )
            # ---- Q = exp(G - E) over the stored region; accum -> partial Z'
            nc.scalar.activation(
                q_sb[:, row_rgn(ib)],
                energy[:, row_rgn(ib)],
                AF.Exp,
                bias=g_sb[:],
                scale=-1.0,
                accum_out=z[:, ib : ib + 1],
            )

        # ---- mirror lower-triangle Q blocks (Q is symmetric); the copy back
        # also row-sums them to complete Z'
        pps = {}
        for ib, jb in MIRROR:
            tps = ps_t.tile([P, 512], FP32R, tag="tps")
            nc.tensor.matmul(
                tps[:, 0:P],
                q_sb[:, jb * 512 + ib * P : jb * 512 + (ib + 1) * P],
                ident[:],
                is_transpose=True,
                start=True,
                stop=True,
            )
            pp = vec_pool.tile([P, 1], FP32, tag=f"pp{ib}{jb}")
            nc.vector.tensor_scalar(
                q_sb[:, ib * 512 + jb * P : ib * 512 + (jb + 1) * P],
                tps[:, 0:P],
                0.0,
                None,
                OP.add,
                OP.add,  # accum reduction op
                accum_out=pp[:],
            )
            pps.setdefault(ib, []).append(pp)

        # ---- s = gamma / Z'
        for ib in range(CB):
            zt = z[:, ib : ib + 1]
            for pp in pps.get(ib, []):
                zn = vec_pool.tile([P, 1], FP32, tag=f"zn{ib}")
                nc.vector.tensor_tensor(zn[:], zt, pp[:], op=OP.add)
                zt = zn[:]
            r = vec_pool.tile([P, 1], FP32, tag=f"r{ib}")
            nc.vector.reciprocal(r[:], zt)
            nc.vector.tensor_tensor(
                s_all[:, ib : ib + 1], r[:], gamma_sb[:], op=OP.mult
            )

        # a few of the next sample's transposes fill the exp tail (copies on
        # DVE: ACT is busy with the exps here)
        if i + 1 < nsamp:
            a_queue = list(range(KB))
            for _ in range(3):
                a_chunk(i + 1, a_queue.pop(0), copy_eng="dve")
        else:
            a_queue = []

        # ---- out = (Q^T-as-stored @ v) * (gamma/Z') + x
        for cb in range(CB):
            n_off = 0
            for nch in NCH:
                if a_queue:
                    # copies on ACT: DVE is busy with the epilogue here
                    a_chunk(i + 1, a_queue.pop(0), copy_eng="act")
                po = ps_o.tile([P, 512], FP32, tag="po")
                for db in range(CB):
                    nc.tensor.matmul(
                        po[:, :nch],
                        q_sb[:, db * 512 + cb * P : db * 512 + (cb + 1) * P],
                        v[:, db * N + n_off : db * N + n_off + nch],
                        start=(db == 0),
                        stop=(db == CB - 1),
                    )
                ot = o_pool.tile([P, 512], FP32, tag="ot")
                nc.vector.scalar_tensor_tensor(
                    ot[:, :nch],
                    po[:, :nch],
                    s_all[:, cb : cb + 1],
                    v[:, cb * N + n_off : cb * N + n_off + nch].bitcast(FP32),
                    op0=OP.mult,
                    op1=OP.add,
                )
                nc.sync.dma_start(
                    out[s, cb * P : (cb + 1) * P, n_off : n_off + nch], ot[:, :nch]
                )
                n_off += nch
        del v_t[i], vt_t[i]

    load_v(0)
    for i in range(nsamp):
        emit(i)


_nc_cache = {}


def _build(reps=1):
    if reps in _nc_cache:
        return _nc_cache[reps]
    nc = bacc.Bacc("TRN2", target_bir_lowering=False, debug=False)
    x_d = nc.dram_tensor("x", [SPC, C, N], FP32, kind="ExternalInput")
    g_d = nc.dram_tensor("gamma", [1], FP32, kind="ExternalInput")
    o_d = nc.dram_tensor("out", [SPC, C, N], FP32, kind="ExternalOutput")
    with tile.TileContext(nc) as tc, ExitStack() as ctx:
        _emit(tc, ctx, x_d.ap(), g_d.ap(), o_d.ap(), reps=reps)
    nc.compile()
    _nc_cache[reps] = nc
    return nc


def _bench_fn(reps, x, gamma):
    """Build a jitted 8-core executor for the reps-times-repeated kernel with
    device-resident inputs.  Used by test.py for differential timing."""
    import jax
    from jax.experimental.shard_map import shard_map
    from jax.sharding import Mesh, NamedSharding, PartitionSpec

    from concourse import bass2jax

    bass2jax.install_neuronx_cc_hook()
    nc = _build(reps=reps)
    pid = nc.partition_id_tensor.name if nc.partition_id_tensor else None
    in_names, out_names, out_avals, zero_outs = [], [], [], []
    for alloc in nc.m.functions[0].allocations:
        if not isinstance(alloc, mybir.MemoryLocationSet):
            continue
        name = alloc.memorylocations[0].name
        if alloc.kind == "ExternalInput":
            if name != pid:
                in_names.append(name)
        elif alloc.kind == "ExternalOutput":
            out_names.append(name)
            shape = tuple(alloc.tensor_shape)
            dtype = mybir.dt.np(alloc.dtype)
            out_avals.append(jax.core.ShapedArray(shape, dtype))
            zero_outs.append(np.zeros(shape, dtype))
    all_in_names = list(in_names) + list(out_names)
    if pid:
        all_in_names.append(pid)

    def _body(*args):
        operands = list(args)
        if pid:
            operands.append(bass2jax.partition_id_tensor())
        return tuple(
            bass2jax._bass_exec_p.bind(
                *operands,
                out_avals=tuple(out_avals),
                in_names=tuple(all_in_names),
                out_names=tuple(out_names),
                lowering_input_output_aliases=(),
                sim_require_finite=True,
                sim_require_nnan=True,
                nc=nc,
            )
        )

    devices = jax.devices()[:NCORES]
    mesh = Mesh(np.asarray(devices), ("core",))
    specs = (PartitionSpec("core"),) * (len(in_names) + len(out_names))
    fn = jax.jit(
        shard_map(
            _body,
            mesh=mesh,
            in_specs=specs,
            out_specs=(PartitionSpec("core"),) * len(out_names),
            check_rep=False,
        ),
        keep_unused=True,
    )
    sh = NamedSharding(mesh, PartitionSpec("core"))
    ins = {
        "x": np.ascontiguousarray(x, dtype=np.float32).reshape(B, C, N),
        "gamma": np.tile(np.ascontiguousarray(gamma, dtype=np.float32), (NCORES,)),
    }
    args = [jax.device_put(ins[n], sh) for n in in_names]
    args += [
        jax.device_put(np.zeros((NCORES * z.shape[0], *z.shape[1:]), z.dtype), sh)
        for z in zero_outs
    ]
    return fn, args


def kernel(x: np.ndarray, gamma: np.ndarray, **run_kwargs) -> np.ndarray:
    assert x.shape == (B, C, H, W), x.shape
    nc = _build()
    xr = np.ascontiguousarray(x, dtype=np.float32).reshape(B, C, N)
    g = np.ascontiguousarray(gamma, dtype=np.float32)
    in_maps = [
        {"x": xr[g_idx * SPC : (g_idx + 1) * SPC], "gamma": g}
        for g_idx in range(NCORES)
    ]
    res = run_bass_kernel_spmd(nc, in_maps, core_ids=list(range(NCORES)), **run_kwargs)
    outs = [res.results[g_idx]["out"] for g_idx in range(NCORES)]
    full = np.concatenate(outs, axis=0).reshape(B, C, H, W).astype(np.float32)
    if run_kwargs:
        kernel.last_results = res
    return full
